# revision 14
# baseline (speedup 1.0000x reference)
"""GCN-GRU cell fused Trainium2 kernel (8-core data parallel).

Math (per batch b):
    A = d * (adj+I).T * d,  d = rowsum(adj+I)^-0.5
    conc1 = [input, hidden]                (N, 65)
    sig   = sigmoid(A @ conc1 @ W1 + b1)   (N, 128)  node-major flat
    r, u  = first/second half of flat(sig) -> pseudo-node split
    rh    = r * hidden_flat
    c     = tanh(A @ [input, rh] @ W2 + b2)
    out   = u * hidden_flat + (1-u) * c

Implementation notes:
  - batch data-parallel: 8 batches per core, 8 cores.
  - Contraction-side d folded into X on host; output-side d applied on
    PSUM->SBUF copy. adj+I is row-permuted on host into even-rows-then-odd
    order (pi) so the GRU pseudo-node remap becomes plain AP slicing.
  - Big A@X GEMMs run in fp8e4 DoubleRow (2x PE rate, contraction 256
    per matmul); small W-GEMMs in bf16. A and X are pre-scaled by
    powers of two (SA, SX) on host to center values in e4m3 range; the
    inverse scale is folded into the drep output-side multiply.
"""

import numpy as np
import ml_dtypes
from contextlib import ExitStack

import concourse.bacc as bacc
import concourse.mybir as mybir
import concourse.tile as tile
from concourse.bass import ts, ds
from concourse.bass_utils import run_bass_kernel_spmd

P = 128
N = 2048
B = 64
H = 64
NCORES = 8
BL = B // NCORES          # 8 batches per core
KT = N // P               # 16 contraction tiles
NT = KT // 2              # 8 (pair-tiles)
CH = N // 512             # 4 output chunks of 512
F32 = mybir.dt.float32
F32R = mybir.dt.float32r
BF16 = mybir.dt.bfloat16
E4 = mybir.dt.float8e4
DR = mybir.MatmulPerfMode.DoubleRow
SA = 32.0     # adjacency fp8 scale (max |a|*SA = 2*32 = 64 < 240)
SX = 256.0    # feature fp8 scale (max |d*h|*SX ~ 44 < 240)
SIG = mybir.ActivationFunctionType.Sigmoid
TANH = mybir.ActivationFunctionType.Tanh

_CACHE = {}


def _build():
    nc = bacc.Bacc("TRN2", target_bir_lowering=False)

    a_d = nc.dram_tensor("a", [N, N], E4, kind="ExternalInput")
    x1_d = nc.dram_tensor("x1", [N, BL * H], E4, kind="ExternalInput")
    # per-kt block padded 8 -> 16 cols: dual-fp8 ldweights needs the
    # pair-dim byte stride 16-aligned
    xin_d = nc.dram_tensor("xin", [P, KT * 2 * BL], E4, kind="ExternalInput")
    hrm_d = nc.dram_tensor("hrm", [BL, N // 2, 2 * H], F32, kind="ExternalInput")
    drep_d = nc.dram_tensor("drep", [P, N], F32, kind="ExternalInput")
    w1h_d = nc.dram_tensor("w1h", [2 * H, 2 * H], BF16, kind="ExternalInput")
    w1i_d = nc.dram_tensor("w1i", [BL + 1, BL, 2 * H], BF16, kind="ExternalInput")
    w2h_d = nc.dram_tensor("w2h", [2 * H, H], BF16, kind="ExternalInput")
    w2i_d = nc.dram_tensor("w2i", [BL + 1, BL, H], BF16, kind="ExternalInput")
    out_d = nc.dram_tensor("out", [BL, N // 2, 2 * H], F32, kind="ExternalOutput")

    out_ap = out_d.ap()

    with tile.TileContext(nc) as tc, ExitStack() as ctx:
        const = ctx.enter_context(tc.tile_pool(name="const", bufs=1))
        x1_sb = const.tile([P, KT, BL * H], E4)
        xin_sb = const.tile([P, KT, 2 * BL], E4)  # [p, kt, b(+pad)], host pre-arranged
        hrm_sb = const.tile([P, BL, NT, 2 * H], F32)
        drep_sb = const.tile([P, N], F32)
        w1h_sb = const.tile([2 * H, 2 * H], BF16)
        w1i_sb = const.tile([BL + 1, BL, 2 * H], BF16)
        w2h_sb = const.tile([2 * H, H], BF16)
        w2i_sb = const.tile([BL + 1, BL, H], BF16)
        sig_r = const.tile([P, NT * BL, 2 * H], BF16)   # slot mt*BL+b, mt 0..7
        sig_u = const.tile([P, NT * BL, 2 * H], BF16)   # slot (mt-8)*BL+b
        x2_sb = const.tile([P, KT, BL * H], E4)
        a_sb = const.tile([P, KT, N], E4)
        axin_sb = const.tile([BL + 1, N], BF16)         # d*(A@input), row=batch; row 8 = ones (bias row)

        x1_r = x1_d.ap().rearrange("(kt p) f -> p kt f", p=P)
        a_r = a_d.ap().rearrange("(kt p) m -> p kt m", p=P)
        # interleaved fine-grained loads for ch0 so the first matmuls start early
        nc.scalar.dma_start(
            xin_sb[:], xin_d.ap().rearrange("p (kt b) -> p kt b", b=2 * BL)
        )
        for g in range(4):
            ks = ts(g, 4)
            nc.sync.dma_start(a_sb[:, ks, 0:512], a_r[:, ks, 0:512])
            nc.scalar.dma_start(x1_sb[:, ks, :], x1_r[:, ks, :])
        for ch in range(1, CH):
            nc.sync.dma_start(
                a_sb[:, :, ds(ch * 512, 512)], a_r[:, :, ds(ch * 512, 512)]
            )
        nc.gpsimd.dma_start(hrm_sb[:], hrm_d.ap().rearrange("b (t p) f -> p b t f", p=P))
        nc.gpsimd.dma_start(drep_sb[:], drep_d.ap())
        nc.sync.dma_start(w1h_sb[:], w1h_d.ap())
        nc.vector.memset(axin_sb[:], 1.0)
        nc.sync.dma_start(w1i_sb[:], w1i_d.ap())
        nc.sync.dma_start(w2h_sb[:], w2h_d.ap())
        nc.sync.dma_start(w2i_sb[:], w2i_d.ap())

        axpool = ctx.enter_context(tc.tile_pool(name="ax", bufs=3))
        cpool = ctx.enter_context(tc.tile_pool(name="c", bufs=2))
        gpool = ctx.enter_context(tc.tile_pool(name="g", bufs=3))
        pps = ctx.enter_context(tc.tile_pool(name="ps", bufs=8, space="PSUM"))

        def big_gemm(ch, xsb, with_in):
            n_ps = 5 if with_in else 4
            ps = [
                pps.tile([P, 512], F32, tag="ps", name=f"ps{i}")
                for i in range(n_ps)
            ]
            for k2 in range(NT):
                rhs = a_sb[:, ds(2 * k2, 2), ds(ch * 512, 512)]
                st, sp = k2 == 0, k2 == NT - 1
                for mf in range(4):
                    nc.tensor.matmul(
                        ps[mf][:],
                        lhsT=xsb[:, ds(2 * k2, 2), ts(mf, P)],
                        rhs=rhs, start=st, stop=sp, perf_mode=DR,
                    )
                if with_in:
                    nc.tensor.matmul(
                        ps[4][:BL],
                        lhsT=xin_sb[:, ds(2 * k2, 2), 0:BL],
                        rhs=rhs, start=st, stop=sp, perf_mode=DR,
                    )
            axf = axpool.tile([P, 4, 512], BF16, tag="ax")
            for mf in range(4):
                nc.vector.tensor_mul(axf[:, mf, :], ps[mf][:], drep_sb[:, ds(ch * 512, 512)])
            if with_in:
                nc.vector.tensor_mul(
                    axin_sb[:BL, ds(ch * 512, 512)], ps[4][:BL],
                    drep_sb[:BL, ds(ch * 512, 512)],
                )
            return axf

        def emit_w1(ch, axf):
            for mt in range(4 * ch, 4 * ch + 4):
                for b in range(BL):
                    pm = pps.tile([P, 512], F32, tag="ps", name="pm")[:, : 2 * H]
                    nc.tensor.matmul(
                        pm[:],
                        lhsT=axf[64 * (b % 2) : 64 * (b % 2) + 64, b // 2, ts(mt % 4, P)],
                        rhs=w1h_sb[64 * (b % 2) : 64 * (b % 2) + 64, :], start=True, stop=False,
                    )
                    nc.tensor.matmul(
                        pm[:],
                        lhsT=axin_sb[:, ds(mt * P, P)],
                        rhs=w1i_sb[:, b, :], start=False, stop=True,
                    )
                    if mt < NT:
                        dst = sig_r[:, mt * BL + b, :]
                    else:
                        dst = sig_u[:, (mt - NT) * BL + b, :]
                    nc.scalar.activation(dst, pm[:], SIG)

        # ---- GCN1 ----  (W1 for chunk ch-1 emitted after big GEMM of ch, so
        # the PE never stalls on the PSUM->SBUF copies feeding W1's lhsT)
        axfs = {}
        for ch in range(CH):
            axfs[ch] = big_gemm(ch, x1_sb, with_in=True)
            if ch >= 1:
                emit_w1(ch - 1, axfs[ch - 1])
        emit_w1(CH - 1, axfs[CH - 1])

        # ---- X2 assembly: x2[p, kt, (b h)] = sig_r-slice * x1-slice ----
        for kt in range(KT):
            te, jo = (kt, 0) if kt < NT else (kt - NT, 64)
            s3 = sig_r[:, ts(te, BL), jo : jo + 64]
            x13 = x1_sb[:, kt, :].rearrange("p (b h) -> p b h", h=H)
            x23 = x2_sb[:, kt, :].rearrange("p (b h) -> p b h", h=H)
            nc.vector.tensor_mul(x23, s3, x13)

        # ---- GCN2 ----
        def emit_w2_gate(ch, axf2):
            for t in (2 * ch, 2 * ch + 1):
                cs = cpool.tile([P, BL, 2 * H], F32, tag="c")
                for b in range(BL):
                    pc = pps.tile([P, 512], F32, tag="ps", name="pc")[:, : 2 * H]
                    for j in (0, 1):
                        lo = 256 * (t % 2) + j
                        nc.tensor.matmul(
                            pc[:, ds(64 * j, 64)],
                            lhsT=axf2[64 * (b % 2) : 64 * (b % 2) + 64, b // 2, lo : lo + 255 : 2],
                            rhs=w2h_sb[64 * (b % 2) : 64 * (b % 2) + 64, :], start=True, stop=False,
                        )
                        nc.tensor.matmul(
                            pc[:, ds(64 * j, 64)],
                            lhsT=axin_sb[:, 256 * t + j : 256 * t + j + 255 : 2],
                            rhs=w2i_sb[:, b, :], start=False, stop=True,
                        )
                    nc.scalar.activation(cs[:, b, :], pc[:], TANH)
                # gate: out = u*(h - c) + c
                u3 = sig_u[:, ts(t, BL), :]
                h3 = hrm_sb[:, :, t, :]
                g = gpool.tile([P, BL, 2 * H], F32, tag="g")
                nc.vector.tensor_sub(g[:], h3, cs[:])
                nc.vector.tensor_mul(g[:], u3, g[:])
                nc.vector.tensor_add(g[:], g[:], cs[:])
                nc.gpsimd.dma_start(
                    out_ap[:, ts(t, P), :].rearrange("b p f -> p b f"), g[:]
                )

        axf2s = {}
        for ch in range(CH):
            axf2s[ch] = big_gemm(ch, x2_sb, with_in=False)
            if ch >= 1:
                emit_w2_gate(ch - 1, axf2s[ch - 1])
        emit_w2_gate(CH - 1, axf2s[CH - 1])

    nc.finalize()
    return nc


def _prep_inputs(input_tensor, hidden, adj, W1, b1, W2, b2):
    f32 = np.float32
    bf16 = ml_dtypes.bfloat16
    e4 = ml_dtypes.float8_e4m3
    input_tensor = np.ascontiguousarray(input_tensor, f32)
    hidden = np.ascontiguousarray(hidden, f32)
    adj = np.ascontiguousarray(adj, f32)

    pi = np.concatenate([np.arange(0, N, 2), np.arange(1, N, 2)])
    deg = 1.0 + adj.sum(axis=1, dtype=np.float64)
    d = (deg ** -0.5).astype(f32)
    a_perm = np.ascontiguousarray(
        (adj + np.eye(N, dtype=f32))[pi] * f32(SA)
    ).astype(e4)

    drep = np.ascontiguousarray(
        np.broadcast_to(d / f32(SA * SX), (P, N)), f32
    )
    w1h = np.ascontiguousarray(np.concatenate([W1[1:], W1[1:]], 0).astype(bf16))
    w1i = np.zeros((BL + 1, BL, 2 * H), bf16)
    for bb in range(BL):
        w1i[bb, bb, :] = W1[0].astype(bf16)
        w1i[BL, bb, :] = b1.astype(bf16)
    w2h = np.ascontiguousarray(np.concatenate([W2[1:], W2[1:]], 0).astype(bf16))
    w2i = np.zeros((BL + 1, BL, H), bf16)
    for bb in range(BL):
        w2i[bb, bb, :] = W2[0].astype(bf16)
        w2i[BL, bb, :] = b2.astype(bf16)

    dh = (d[None, :, None] * f32(SX)) * hidden   # (B, N, H), fp8-scaled
    din = (d[None, :] * f32(SX)) * input_tensor  # (B, N), fp8-scaled

    in_maps = []
    for c in range(NCORES):
        bs = slice(BL * c, BL * c + BL)
        x1 = np.ascontiguousarray(
            dh[bs][:, pi, :].transpose(1, 0, 2).reshape(N, BL * H)
        ).astype(e4)
        xin8 = din[bs][:, pi].T.reshape(KT, P, BL).transpose(1, 0, 2)  # (P, KT, BL)
        xin = np.zeros((P, KT, 2 * BL), e4)
        xin[:, :, :BL] = xin8.astype(e4)
        xin = xin.reshape(P, KT * 2 * BL)
        hrm = np.ascontiguousarray(hidden[bs].reshape(BL, N // 2, 2 * H))
        in_maps.append({
            "a": a_perm, "x1": x1, "xin": xin, "hrm": hrm, "drep": drep,
            "w1h": w1h, "w1i": w1i, "w2h": w2h, "w2i": w2i,
        })
    return in_maps


LAST_RESULTS = None


def kernel(input_tensor, hidden, adj, W1, b1, W2, b2):
    global LAST_RESULTS
    if "nc" not in _CACHE:
        _CACHE["nc"] = _build()
    nc = _CACHE["nc"]
    in_maps = _prep_inputs(input_tensor, hidden, adj, W1, b1, W2, b2)
    res = run_bass_kernel_spmd(nc, in_maps, core_ids=list(range(NCORES)))
    LAST_RESULTS = res
    outs = [r["out"] for r in res.results]
    return np.concatenate(outs, axis=0).reshape(B, N, H).astype(np.float32)


if __name__ == "__main__":
    rng = np.random.default_rng(0)
    inputs = {
        "input_tensor": rng.standard_normal((B, N), dtype=np.float32),
        "hidden": rng.standard_normal((B, N, H), dtype=np.float32),
        "adj": rng.random((N, N), dtype=np.float32),
        "W1": rng.standard_normal((H + 1, 2 * H), dtype=np.float32) * 0.15,
        "b1": np.full((2 * H,), 0.4, np.float32),
        "W2": rng.standard_normal((H + 1, H), dtype=np.float32) * 0.15,
        "b2": np.full((H,), 0.6, np.float32),
    }
    out = kernel(**inputs)
    print(out.shape, out.dtype)

